# revision 4
# baseline (speedup 1.0000x reference)
"""Trainium2 Bass kernel for GQA attention (B=2, S=2048, D=2048, H=32, KVH=8).

Sharding: batch data-parallel across 2 groups of 4 cores; within a group,
4-way tensor parallel over heads (8 q heads + their 2 kv heads per core).
One device-side bf16 ReduceScatter(add) per 512-token chunk over each 4-core
group after the wo matmul, written directly into the output tensor; the host
concatenates the token slices.

The device program is identical on all 8 cores (SPMD); all per-core
variation (batch slice, head slice) is carried by the input data.

v3 structure: a software pipeline over 512-token chunks —
   proj(c) -> rope(c) -> kTrep/v(c) -> attention(c) -> wo(c) -> RS(c)
All matmuls run in bf16 (fp32 PSUM accum).  Changes vs v2:
 - All host->device layouts are partition-major contiguous, so every load is
   one descriptor run per partition (the v2 rearrange loads cost ~77us of
   sync-engine descriptor generation at startup).
 - wo path is bf16 end to end (v2's fp32r ran in FP32-HIGH mode at ~1.5x
   the bf16 matmul cost); the collective also moves bf16 (half the bytes).
 - One RS per chunk, output written directly to y_out: no y_rs staging copy
   (v2's final copies waited on collectives from the sync queue and
   head-of-line-blocked the outc copies wo needed, a ~40us stall).
 - Causal N-shrink: diagonal tiles only compute/exp/AV the unmasked
   column range [128r, 512).
 - PSUM->SBUF copies moved off the scalar engine so it runs pure Exp
   (no activation-table swaps).

Layout notes:
 - Host passes x pre-transposed and chunk-major: xT[c][p][k*512+n] =
   x[b, c*512+n, k*128+p]; every matmul consumes it directly.
 - wq/wk columns are permuted on host into an "even dims block / odd dims
   block" (A/B) layout so RoPE is full-partition DVE work; wq carries the
   1/sqrt(HD) scale (exact power of two).
 - Scores are computed transposed (scoresT[sk, sq]) so probsT feeds the AV
   matmul directly with no transposes in the attention path.
 - Causal mask on diagonal tiles: one extra accumulating matmul,
   LEones[k,p]=[k<=p] x shifted -1e9 one-hot diagonal.
 - Softmax denominators ride along as a ones column in v (M=65 AV matmul);
   normalization multiplies by the partition-broadcast reciprocal.
"""

import os
import sys
import functools

import numpy as np

if "/opt/trn_rl_repo" not in sys.path:
    sys.path.insert(0, "/opt/trn_rl_repo")

B, S, D = 2, 2048, 2048
H, KVH = 32, 8
HD = D // H            # 64
N_CORES = 8
GROUP = 4              # cores per batch group (tensor parallel width)
HPC = 8                # query heads per core
KVPC = 2               # kv heads per core
SQC = 512              # sq chunk (psum bank width in fp32)
PT = 128               # partition tile
KT = D // PT           # 16 contraction tiles
NT = S // PT           # 16 token tiles
NCHUNK = S // SQC      # 4
TPC = SQC // PT        # tok tiles per chunk (4)
NEG = -1e9


def _build_program():
    import concourse.bass as bass
    import concourse.bacc as bacc
    import concourse.mybir as mybir
    import concourse.tile as tile
    import ml_dtypes
    from contextlib import ExitStack

    f32 = mybir.dt.float32
    bf16 = mybir.dt.bfloat16

    nc = bacc.Bacc("TRN2", target_bir_lowering=False, debug=False,
                   num_devices=N_CORES)

    # ---- dram parameters (all partition-major contiguous) ----------------
    xT_d = nc.dram_tensor("xt", [NCHUNK, PT, KT * SQC], bf16,
                          kind="ExternalInput")
    wq_d = nc.dram_tensor("wq", [PT, KT * HPC * HD], bf16,
                          kind="ExternalInput")
    wk_d = nc.dram_tensor("wk", [PT, KT * KVPC * HD], bf16,
                          kind="ExternalInput")
    wv_d = nc.dram_tensor("wv", [PT, KT * KVPC * HD], bf16,
                          kind="ExternalInput")
    wo_d = nc.dram_tensor("wo", [PT, TPC * D], bf16, kind="ExternalInput")
    cos_d = nc.dram_tensor("cosr", [PT, S], bf16, kind="ExternalInput")
    sin_d = nc.dram_tensor("sinr", [PT, S], bf16, kind="ExternalInput")
    y_out = nc.dram_tensor("y", [S // GROUP, D], bf16, kind="ExternalOutput")

    y_part = nc.dram_tensor("y_part", [S, D], bf16)
    y_rs = nc.dram_tensor("y_rs", [S // GROUP, D], bf16)

    # ---- inline constants ------------------------------------------------
    leones = np.zeros((PT, PT), np.float32)      # leones[k, p] = 1 if k <= p
    for k in range(PT):
        leones[k, k:] = 1.0
    # col i: -1e9 one-hot at k = i+1 (masks p > i within the diagonal tile)
    dmast = np.zeros((PT, SQC), np.float32)
    for i in range(PT - 1):
        dmast[i + 1, i] = NEG
    ident = np.eye(PT, dtype=ml_dtypes.bfloat16)
    ones1 = np.ones((PT, 1), ml_dtypes.bfloat16)

    le_d = nc.inline_tensor(leones.astype(ml_dtypes.bfloat16), "leones")
    dm_d = nc.inline_tensor(dmast.astype(ml_dtypes.bfloat16), "dmaster")
    id_d = nc.inline_tensor(ident, "ident")
    on_d = nc.inline_tensor(ones1, "ones1")

    Exp = mybir.ActivationFunctionType.Exp
    groups = [[0, 1, 2, 3], [4, 5, 6, 7]]

    with tile.TileContext(nc) as tc, ExitStack() as ctx:
        keep = ctx.enter_context(tc.tile_pool(name="keep", bufs=1))
        # packed K cache: krp[kv] rows = [kv(a32 b32); kv(a32 b32)] replicated
        krp0 = keep.tile([PT, S], bf16)
        krp1 = keep.tile([PT, S], bf16)
        krp = [krp0, krp1]
        v_sb = keep.tile([PT, KVPC, NT, HD + 1], bf16)   # col 64 = ones
        cos_sb = keep.tile([PT, S], bf16)
        sin_sb = keep.tile([PT, S], bf16)
        le_sb = keep.tile([PT, PT], bf16)
        dm_sb = keep.tile([PT, SQC], bf16)
        id_sb = keep.tile([PT, PT], bf16)
        wq_sb = keep.tile([PT, KT, HPC * HD], bf16)
        wk_sb = keep.tile([PT, KT, KVPC * HD], bf16)
        wv_sb = keep.tile([PT, KT, KVPC * HD], bf16)
        wo_sb = keep.tile([PT, TPC, D], bf16)

        xcache = {}
        qcps = {}

        xpool = ctx.enter_context(tc.tile_pool(name="xp", bufs=3))
        qpool = ctx.enter_context(tc.tile_pool(name="qp", bufs=2))
        qppool = ctx.enter_context(tc.tile_pool(name="qpp", bufs=2))
        kpool = ctx.enter_context(tc.tile_pool(name="kp", bufs=2))
        vtp = ctx.enter_context(tc.tile_pool(name="vtp", bufs=2))
        otp = ctx.enter_context(tc.tile_pool(name="otp", bufs=2))
        rtmp = ctx.enter_context(tc.tile_pool(name="rtmp", bufs=1))
        probs = ctx.enter_context(tc.tile_pool(name="probs", bufs=8))
        bcp = ctx.enter_context(tc.tile_pool(name="bcp", bufs=2))
        rcp = ctx.enter_context(tc.tile_pool(name="rcp", bufs=2))
        osg = ctx.enter_context(tc.tile_pool(name="osg", bufs=2))
        ysb = ctx.enter_context(tc.tile_pool(name="ysb", bufs=3))
        mw = ctx.enter_context(tc.tile_pool(name="mw", bufs=2, space="PSUM"))
        sps = ctx.enter_context(tc.tile_pool(name="sps", bufs=4, space="PSUM"))
        aps = ctx.enter_context(tc.tile_pool(name="aps", bufs=2, space="PSUM"))

        def load_x(c):
            if c >= NCHUNK or c in xcache:
                return
            xt = xpool.tile([PT, KT, SQC], bf16, tag="xt", name=f"xt{c}")
            nc.sync.dma_start(
                out=xt[:],
                in_=xT_d[c].rearrange("p (k n) -> p k n", k=KT))
            xcache[c] = xt

        nc.sync.dma_start(out=wq_sb[:],
                          in_=wq_d.ap().rearrange("p (k n) -> p k n", k=KT))
        load_x(0)
        nc.sync.dma_start(out=wk_sb[:],
                          in_=wk_d.ap().rearrange("p (k n) -> p k n", k=KT))
        nc.sync.dma_start(out=wv_sb[:],
                          in_=wv_d.ap().rearrange("p (k n) -> p k n", k=KT))
        nc.sync.dma_start(out=cos_sb[:], in_=cos_d[:])
        nc.sync.dma_start(out=sin_sb[:], in_=sin_d[:])
        nc.sync.dma_start(out=le_sb[:], in_=le_d[:])
        nc.sync.dma_start(out=dm_sb[:], in_=dm_d[:])
        nc.sync.dma_start(out=id_sb[:], in_=id_d[:])
        # ones column of v (every (kv, t) slot)
        ones_src = bass.AP(tensor=on_d.ap().tensor, offset=0,
                           ap=[[1, PT], [0, KVPC * NT], [1, 1]])
        vcol = v_sb[:, :, :, HD:HD + 1]
        ones_dst = bass.AP(tensor=vcol.tensor, offset=vcol.offset,
                           ap=[list(vcol.ap[0]), [HD + 1, KVPC * NT], [1, 1]])
        nc.sync.dma_start(out=ones_dst, in_=ones_src)
        load_x(1)
        nc.sync.dma_start(out=wo_sb[:],
                          in_=wo_d.ap().rearrange("p (k n) -> p k n", k=TPC))

        def rope_pair(a, b, cs, sn, nm):
            """a' = a*cos - b*sin ; b' = a*sin + b*cos (bf16, in place)."""
            t1 = rtmp.tile(a.shape, bf16, tag="t1", name=f"t1{nm}")
            t2 = rtmp.tile(a.shape, bf16, tag="t2", name=f"t2{nm}")
            t3 = rtmp.tile(a.shape, bf16, tag="t3", name=f"t3{nm}")
            nc.vector.tensor_mul(t1[:], a, cs)
            nc.vector.tensor_mul(t2[:], a, sn)
            nc.vector.tensor_mul(t3[:], b, sn)
            nc.vector.tensor_sub(a, t1[:], t3[:])
            t4 = rtmp.tile(a.shape, bf16, tag="t3", name=f"t4{nm}")
            nc.vector.tensor_mul(t4[:], b, cs)
            nc.vector.tensor_add(b, t2[:], t4[:])

        def prep(c):
            csl = slice(c * SQC, (c + 1) * SQC)
            load_x(c)
            load_x(c + 1)          # prefetch next chunk behind this one
            xt = xcache.pop(c)

            qc = qpool.tile([PT, 4, SQC], bf16, tag="qc", name=f"qc{c}")
            kc = kpool.tile([PT, SQC], bf16, tag="kc", name=f"kc{c}")
            vtc = vtp.tile([PT, SQC], bf16, tag="vtc", name=f"vtc{c}")
            for mt in range(4):
                ps = mw.tile([PT, SQC], f32, tag="ps", name=f"qps{c}_{mt}")
                for k in range(KT):
                    nc.tensor.matmul(
                        ps[:], wq_sb[:, k, mt * PT:(mt + 1) * PT],
                        xt[:, k, :],
                        start=(k == 0), stop=(k == KT - 1))
                nc.vector.tensor_copy(qc[:, mt, :], ps[:])
            for dst, wsb, nm in ((kc, wk_sb, "k"), (vtc, wv_sb, "v")):
                ps = mw.tile([PT, SQC], f32, tag="ps", name=f"ps{nm}{c}")
                for k in range(KT):
                    nc.tensor.matmul(
                        ps[:], wsb[:, k, :],
                        xt[:, k, :],
                        start=(k == 0), stop=(k == KT - 1))
                nc.vector.tensor_copy(dst[:], ps[:])

            # ---- rope(c) ------------------------------------------------
            for j in range(2):
                rope_pair(qc[:, j, :], qc[:, 2 + j, :],
                          cos_sb[:, csl], sin_sb[:, csl], f"q{c}_{j}")
            # k pair: rows 0:64 / 64:128 — stage B rows to base 0 via DMA
            bst = rtmp.tile([64, SQC], bf16, tag="t1", name=f"bst{c}")
            nc.sync.dma_start(out=bst[:], in_=kc[64:128, :])
            kt1 = rtmp.tile([64, SQC], bf16, tag="t2", name=f"kt1{c}")
            kt2 = rtmp.tile([64, SQC], bf16, tag="t3", name=f"kt2{c}")
            kt3 = rtmp.tile([64, SQC], bf16, tag="t1b", name=f"kt3{c}")
            kt4 = rtmp.tile([64, SQC], bf16, tag="t2b", name=f"kt4{c}")
            nc.vector.tensor_mul(kt1[:], kc[0:64, :], cos_sb[0:64, csl])
            nc.vector.tensor_mul(kt2[:], kc[0:64, :], sin_sb[0:64, csl])
            nc.vector.tensor_mul(kt3[:], bst[:], sin_sb[0:64, csl])
            nc.vector.tensor_mul(kt4[:], bst[:], cos_sb[0:64, csl])
            nc.vector.tensor_sub(kc[0:64, :], kt1[:], kt3[:])
            kbr = rtmp.tile([64, SQC], bf16, tag="t3b", name=f"kbr{c}")
            nc.vector.tensor_add(kbr[:], kt2[:], kt4[:])
            nc.sync.dma_start(out=kc[64:128, :], in_=kbr[:])

            # ---- pack(c): head-contiguous q (K=64 scores) ---------------
            # qcp[j] rows: head 2j at [0:64] (a32 b32), head 2j+1 at [64:128]
            qcp = qppool.tile([PT, 4, SQC], bf16, tag="qcp", name=f"qcp{c}")
            qcps[c] = qcp
            for qh in range(HPC):
                so = slice((qh % 4) * 32, (qh % 4) * 32 + 32)
                do = (qh % 2) * 64
                nc.sync.dma_start(out=qcp[do:do + 32, qh // 2, :],
                                  in_=qc[so, qh // 4, :])
                nc.sync.dma_start(out=qcp[do + 32:do + 64, qh // 2, :],
                                  in_=qc[so, 2 + qh // 4, :])
            # krp[kv] rows [0:64]=[64:128] = kv's (a32 b32)
            for kv in range(KVPC):
                for rep in range(2):
                    ro = rep * 64
                    nc.sync.dma_start(
                        out=krp[kv][ro:ro + 32, csl],
                        in_=kc[kv * 32:(kv + 1) * 32, :])
                    nc.sync.dma_start(
                        out=krp[kv][ro + 32:ro + 64, csl],
                        in_=kc[64 + kv * 32:64 + (kv + 1) * 32, :])

            # ---- v(c): transpose vT chunk into v_sb ---------------------
            for tl in range(TPC):
                t = c * TPC + tl
                tp = mw.tile([PT, SQC], f32, tag="ps", name=f"tp{c}_{tl}")
                tpb = tp[:, 0:PT].bitcast(bf16)[:, 0:PT]
                nc.tensor.transpose(tpb,
                                    vtc[:, tl * PT:(tl + 1) * PT],
                                    id_sb[:])
                nc.vector.tensor_copy(v_sb[:, 0, t, 0:HD], tpb[:, 0:HD])
                nc.vector.tensor_copy(v_sb[:, 1, t, 0:HD], tpb[:, HD:2 * HD])

        prep(0)
        for c in range(NCHUNK):
            if c + 1 < NCHUNK:
                prep(c + 1)

            # ---- attention(c) -------------------------------------------
            qcp = qcps.pop(c)
            outc = otp.tile([PT, 4, SQC], bf16, tag="outc", name=f"outc{c}")
            ntk = 4 * c + 4
            LAG = 2
            for qh in range(HPC):
                    g = qh // 4            # kv group
                    base = (qh % 2) * 64
                    av = aps.tile([PT, SQC], f32, tag="av",
                                  name=f"av{c}_{qh}")
                    pbq = []
                    offs = []
                    for tt in range(ntk + LAG):
                        if tt < ntk:
                            t = tt
                            ksl = slice(t * PT, (t + 1) * PT)
                            # causal N-shrink: diagonal tile r only covers
                            # sq columns [128r, 512)
                            diag = t >= 4 * c
                            off = (t - 4 * c) * PT if diag else 0
                            sc = sps.tile([PT, SQC], f32, tag="sc",
                                          name=f"sc{c}_{qh}_{t}")
                            nc.tensor.matmul(
                                sc[:, off:], krp[g][base:base + 64, ksl],
                                qcp[base:base + 64, qh // 2, off:],
                                start=True, stop=not diag,
                                tile_position=(base, 0))
                            if diag:
                                nc.tensor.matmul(
                                    sc[:, off:], le_sb[:],
                                    dm_sb[:, 0:SQC - off],
                                    start=False, stop=True)
                            pb = probs.tile([PT, SQC], bf16, tag="pb",
                                            name=f"pb{c}_{qh}_{t}")
                            nc.scalar.activation(pb[:, off:], sc[:, off:],
                                                 Exp)
                            pbq.append(pb)
                            offs.append(off)
                        if tt >= LAG:
                            t = tt - LAG
                            off = offs[t]
                            nc.tensor.matmul(
                                av[0:HD + 1, off:], v_sb[:, g, t, :],
                                pbq[t][:, off:],
                                start=(t == 0), stop=(t == ntk - 1))
                    rc = rcp.tile([1, SQC], f32, tag="rc",
                                  name=f"rc{c}_{qh}")
                    nc.vector.reciprocal(rc[:], av[HD:HD + 1, :])
                    bc = bcp.tile([64, SQC], f32, tag="bc",
                                  name=f"bc{c}_{qh}")
                    nc.gpsimd.partition_broadcast(bc[:], rc[:])
                    dst = outc[(qh % 2) * HD:(qh % 2 + 1) * HD, qh // 2, :]
                    if qh % 2 == 0:
                        nc.vector.tensor_mul(dst, av[0:HD, :], bc[:])
                    else:
                        st = osg.tile([64, SQC], bf16, tag="st",
                                      name=f"st{c}_{qh}")
                        nc.vector.tensor_mul(st[:], av[0:HD, :], bc[:])
                        nc.sync.dma_start(out=dst, in_=st[:])

            # ---- wo(c) --------------------------------------------------
            for tl in range(TPC):
                tt = c * TPC + tl
                yt = ysb.tile([PT, D], bf16, tag="yt", name=f"yt{c}_{tl}")
                for nk in range(4):
                    yp = mw.tile([PT, SQC], f32, tag="ps",
                                 name=f"yp{c}_{tl}_{nk}")
                    for k4 in range(4):
                        nc.tensor.matmul(
                            yp[:], outc[:, k4, tl * PT:(tl + 1) * PT],
                            wo_sb[:, k4, nk * SQC:(nk + 1) * SQC],
                            start=(k4 == 0), stop=(k4 == 3))
                    nc.vector.tensor_copy(yt[:, nk * SQC:(nk + 1) * SQC],
                                          yp[:])
                nc.sync.dma_start(out=y_part[tt * PT:(tt + 1) * PT, :],
                                  in_=yt[:])

            # ---- RS(c): one bf16 reduce-scatter per chunk ---------------
            # The y_rs -> y_out copy for chunk c-1 is issued on the gpsimd
            # queue right AFTER this chunk's RS trigger: straight-line
            # collective ordering means RS(c-1) has completed by the time
            # the trigger fires, so the copy's wait never blocks the queue
            # (the broadcasts behind it flow freely).
            nc.gpsimd.collective_compute(
                "ReduceScatter", mybir.AluOpType.add,
                replica_groups=groups,
                ins=[y_part.ap()[c * SQC:(c + 1) * SQC, :]],
                outs=[y_rs.ap()[c * PT:(c + 1) * PT, :]])
            if c > 0:
                nc.gpsimd.dma_start(
                    out=y_out.ap()[(c - 1) * PT:c * PT, :],
                    in_=y_rs.ap()[(c - 1) * PT:c * PT, :])

        nc.gpsimd.dma_start(
            out=y_out.ap()[(NCHUNK - 1) * PT:NCHUNK * PT, :],
            in_=y_rs.ap()[(NCHUNK - 1) * PT:NCHUNK * PT, :])

    nc.compile()
    return nc


@functools.lru_cache(maxsize=2)
def _get_program():
    return _build_program()


def _host_inputs(x, wq, wk, wv, wo, cos, sin):
    """Build the 8 per-core input maps (all partition-major contiguous)."""
    import ml_dtypes

    perm_q = np.empty(HPC * HD, np.int64)
    for rho in range(HPC * HD):
        blk, rem = divmod(rho, HPC * HD // 2)
        h, i = divmod(rem, 32)
        perm_q[rho] = h * HD + 2 * i + blk
    perm_k = np.empty(KVPC * HD, np.int64)
    for rho in range(KVPC * HD):
        blk, rem = divmod(rho, KVPC * HD // 2)
        kv, i = divmod(rem, 32)
        perm_k[rho] = kv * HD + 2 * i + blk

    reps = np.tile(np.arange(32), 4)
    cosr = np.ascontiguousarray(cos.T[reps]).astype(ml_dtypes.bfloat16)
    sinr = np.ascontiguousarray(sin.T[reps]).astype(ml_dtypes.bfloat16)

    def pmajor(w):
        """[D_in, M] -> [128, KT_w * M] with [p, k*M+m] = w[k*128+p, m]."""
        kt = w.shape[0] // PT
        return np.ascontiguousarray(
            w.reshape(kt, PT, w.shape[1]).transpose(1, 0, 2)
            .reshape(PT, kt * w.shape[1])).astype(ml_dtypes.bfloat16)

    xts = []
    for b in range(B):
        # [c, p, k*512+n] = x[b, c*512+n, k*128+p]
        xb = x[b].reshape(NCHUNK, SQC, KT, PT).transpose(0, 3, 2, 1)
        xts.append(np.ascontiguousarray(
            xb.reshape(NCHUNK, PT, KT * SQC)).astype(ml_dtypes.bfloat16))

    scale = np.float32(1.0 / np.sqrt(HD))
    in_maps = []
    for core in range(N_CORES):
        b, hg = divmod(core, GROUP)
        qcols = slice(hg * HPC * HD, (hg + 1) * HPC * HD)
        kcols = slice(hg * KVPC * HD, (hg + 1) * KVPC * HD)
        wq_c = (wq[:, qcols] * scale)[:, perm_q]
        wk_c = wk[:, kcols][:, perm_k]
        wv_c = wv[:, kcols]
        wo_c = wo[qcols, :]
        in_maps.append({
            "xt": xts[b],
            "wq": pmajor(wq_c),
            "wk": pmajor(wk_c),
            "wv": pmajor(wv_c),
            "wo": pmajor(wo_c),
            "cosr": cosr,
            "sinr": sinr,
        })
    return in_maps


def _assemble(results):
    """results[core]["y"]: [S/GROUP, D] bf16; chunk c rows [c*128:(c+1)*128]
    hold tokens c*512 + r*128 .. +128 for group rank r."""
    out = np.empty((B, S, D), np.float32)
    for b in range(B):
        for r in range(GROUP):
            y = np.asarray(results[b * GROUP + r]["y"], np.float32)
            for c in range(NCHUNK):
                rows = slice(c * SQC + r * PT, c * SQC + (r + 1) * PT)
                out[b, rows, :] = y[c * PT:(c + 1) * PT, :]
    return out


def _is_causal(mask):
    if mask.shape != (S, S):
        return False
    expect = np.where(np.tril(np.ones((S, S), bool)), np.float32(0.0),
                      np.float32(NEG))
    return np.array_equal(mask, expect)


def _numpy_fallback(x, wq, wk, wv, wo, cos, sin, mask):
    """Exact reference math on host (only used if mask isn't causal)."""
    xq = (x @ wq).reshape(B, S, H, HD)
    xk = (x @ wk).reshape(B, S, KVH, HD)
    xv = (x @ wv).reshape(B, S, KVH, HD)

    def rope(t):
        tr = t.reshape(*t.shape[:-1], HD // 2, 2)
        a, b = tr[..., 0], tr[..., 1]
        c = cos[None, :, None, :]
        s_ = sin[None, :, None, :]
        out = np.stack([a * c - b * s_, a * s_ + b * c], axis=-1)
        return out.reshape(t.shape)

    xq, xk = rope(xq), rope(xk)
    xk = np.repeat(xk, H // KVH, axis=2)
    xv = np.repeat(xv, H // KVH, axis=2)
    q = xq.transpose(0, 2, 1, 3)
    k = xk.transpose(0, 2, 1, 3)
    v = xv.transpose(0, 2, 1, 3)
    sc = np.einsum("bhqd,bhkd->bhqk", q, k) / np.sqrt(np.float32(HD))
    sc = sc + mask[None, None]
    sc = sc - sc.max(-1, keepdims=True)
    p = np.exp(sc)
    p /= p.sum(-1, keepdims=True)
    out = np.einsum("bhqk,bhkd->bhqd", p, v)
    out = out.transpose(0, 2, 1, 3).reshape(B, S, H * HD)
    return (out @ wo).astype(np.float32)


def _ensure_ntff_hook():
    """Provide antenv.axon_hooks (missing on this image) so trace=True works."""
    try:
        from antenv.axon_hooks import get_axon_ntff_profile_hook  # noqa: F401
        return True
    except ImportError:
        pass
    try:
        import types
        import antenv
        from trn_agent_boot.trn_boot import _ntff_profile_via_ctypes

        mod = types.ModuleType("antenv.axon_hooks")
        _state = {"hook": None}
        mod.set_axon_ntff_profile_hook = \
            lambda h: _state.__setitem__("hook", h)
        mod.get_axon_ntff_profile_hook = lambda: _state["hook"]
        sys.modules["antenv.axon_hooks"] = mod
        antenv.axon_hooks = mod
        mod.set_axon_ntff_profile_hook(
            _ntff_profile_via_ctypes("/opt/axon/libaxon_pjrt.so"))
        return mod.get_axon_ntff_profile_hook() is not None
    except Exception:
        return False


def kernel(x, wq, wk, wv, wo, cos, sin, mask):
    x = np.asarray(x, np.float32)
    wq = np.asarray(wq, np.float32)
    wk = np.asarray(wk, np.float32)
    wv = np.asarray(wv, np.float32)
    wo = np.asarray(wo, np.float32)
    cos = np.asarray(cos, np.float32)
    sin = np.asarray(sin, np.float32)
    mask = np.asarray(mask, np.float32)

    if not _is_causal(mask):
        return _numpy_fallback(x, wq, wk, wv, wo, cos, sin, mask)

    from concourse.bass_utils import run_bass_kernel_spmd

    nc = _get_program()
    in_maps = _host_inputs(x, wq, wk, wv, wo, cos, sin)
    trace = bool(int(os.environ.get("ATTN_TRACE", "0")))
    if trace and not _ensure_ntff_hook():
        trace = False
    res = run_bass_kernel_spmd(nc, in_maps, core_ids=list(range(N_CORES)),
                               trace=trace)
    if trace:
        kernel.last_exec_time_ns = res.exec_time_ns
        kernel.last_results = res
    return _assemble(res.results)


# revision 7
# speedup vs baseline: 1.0499x; 1.0499x over previous
"""Trainium2 Bass kernel for GQA attention (B=2, S=2048, D=2048, H=32, KVH=8).

Sharding: batch data-parallel across 2 groups of 4 cores; within a group,
4-way tensor parallel over heads (8 q heads + their 2 kv heads per core).
One device-side bf16 ReduceScatter(add) per 512-token chunk over each 4-core
group after the wo matmul; the host concatenates the token slices.

The device program is identical on all 8 cores (SPMD); all per-core
variation (batch slice, head slice) is carried by the input data.

v4 structure: the attention inner loop processes HEAD PAIRS with the two
score matmuls issued back-to-back into disjoint PE row groups (rows 0:63 /
64:127), so they execute concurrently on the 128x128 array.  Between
attention steps a "filler pump" interleaves matmuls from the next chunk's
projections and the previous chunk's wo into the PE queue, filling the
exp-wait gaps so the tensor engine stays dense (HAM stays at 2.4 GHz).

 - prep(c) (projections+rope+packing) runs as filler inside attention(c-1);
   wo(c-1) runs as filler inside attention(c); the ReduceScatter for chunk
   c-1 triggers as soon as its wo filler drains.
 - q/k repacking (rope A/B-block layout -> per-head score layout) is done
   with small permutation-matrix matmuls on the PE instead of 24 SBUF-SBUF
   DMAs per chunk (which serialized on the sync queue for ~15us/chunk).
 - The causal mask is applied by multiplying the probs of diagonal tiles
   with a 0/1 triangle on the vector engine (no -1e9 matmul on the PE), and
   diagonal tiles only compute/exp/AV the live column range [128r, 512).
 - The scalar engine runs pure Exp (activation-table stays loaded); all
   PSUM->SBUF copies are on the vector engine.
 - All host->device layouts are partition-major contiguous; weight/x loads
   are split in halves so the first projection matmul starts ~3us in.

Layout notes:
 - Host passes x pre-transposed and chunk-major: xT[c][p][k*512+n] =
   x[b, c*512+n, k*128+p].
 - wq/wk columns are permuted on host into an "even dims block / odd dims
   block" (A/B) layout so RoPE is full-partition DVE work; wq carries the
   1/sqrt(HD) scale (exact power of two).
 - Scores are computed transposed (scoresT[sk, sq]) so probsT feeds the AV
   matmul directly with no transposes in the attention path.
 - Softmax denominators ride along as a ones column in v (M=65 AV matmul);
   normalization multiplies by the partition-broadcast reciprocal.
"""

import os
import sys
import functools

import numpy as np

if "/opt/trn_rl_repo" not in sys.path:
    sys.path.insert(0, "/opt/trn_rl_repo")

B, S, D = 2, 2048, 2048
H, KVH = 32, 8
HD = D // H            # 64
N_CORES = 8
GROUP = 4              # cores per batch group (tensor parallel width)
HPC = 8                # query heads per core
KVPC = 2               # kv heads per core
SQC = 512              # sq chunk (psum bank width in fp32)
PT = 128               # partition tile
KT = D // PT           # 16 contraction tiles
NT = S // PT           # 16 token tiles
NCHUNK = S // SQC      # 4
TPC = SQC // PT        # tok tiles per chunk (4)
NEG = -1e9
LAG = 2                # exp -> AV pipeline depth, in attention steps
PUMP = 2               # filler matmuls pumped per attention step


def _build_program():
    import concourse.bass as bass
    import concourse.bacc as bacc
    import concourse.mybir as mybir
    import concourse.tile as tile
    import ml_dtypes
    from contextlib import ExitStack

    f32 = mybir.dt.float32
    bf16 = mybir.dt.bfloat16

    nc = bacc.Bacc("TRN2", target_bir_lowering=False, debug=False,
                   num_devices=N_CORES)

    # ---- dram parameters (all partition-major contiguous) ----------------
    xT_d = nc.dram_tensor("xt", [NCHUNK, PT, KT * SQC], bf16,
                          kind="ExternalInput")
    wq_d = nc.dram_tensor("wq", [PT, KT * HPC * HD], bf16,
                          kind="ExternalInput")
    wk_d = nc.dram_tensor("wk", [PT, KT * KVPC * HD], bf16,
                          kind="ExternalInput")
    wv_d = nc.dram_tensor("wv", [PT, KT * KVPC * HD], bf16,
                          kind="ExternalInput")
    wo_d = nc.dram_tensor("wo", [PT, TPC * D], bf16, kind="ExternalInput")
    cos_d = nc.dram_tensor("cosr", [PT, S], bf16, kind="ExternalInput")
    sin_d = nc.dram_tensor("sinr", [PT, S], bf16, kind="ExternalInput")
    y_out = nc.dram_tensor("y", [S // GROUP, D], bf16, kind="ExternalOutput")

    y_part = nc.dram_tensor("y_part", [S, D], bf16)
    y_rs = nc.dram_tensor("y_rs", [S // GROUP, D], bf16)

    # ---- inline constants ------------------------------------------------
    # TRI[p, i] = 1 if p <= i else 0  (keep-mask for diagonal tiles)
    tri = np.zeros((PT, SQC), np.float32)
    for p in range(PT):
        tri[p, p:] = 1.0
    ident = np.eye(PT, dtype=ml_dtypes.bfloat16)
    ones1 = np.ones((PT, 1), ml_dtypes.bfloat16)
    # qcp pack permutations: qcp col j (pair 2j,2j+1) =
    #   PA[j%2].T @ qc[:, j//2] + PB[j%2].T @ qc[:, 2 + j//2]
    pmats = np.zeros((PT, 4, PT), np.float32)   # [src, {PA0,PA1,PB0,PB1}, dst]
    for m in range(2):
        for i in range(32):
            pmats[64 * m + i, m, i] = 1.0            # PA_m: a -> [0:32]
            pmats[64 * m + 32 + i, m, 64 + i] = 1.0  # PA_m: a2 -> [64:96]
            pmats[64 * m + i, 2 + m, 32 + i] = 1.0   # PB_m: b -> [32:64]
            pmats[64 * m + 32 + i, 2 + m, 96 + i] = 1.0
    # krp pack: krp[kv] = PK[kv].T @ kc  (kv dims a/b stacked, duplicated)
    kmats = np.zeros((PT, KVPC, PT), np.float32)
    for kv in range(KVPC):
        for i in range(32):
            kmats[32 * kv + i, kv, i] = 1.0
            kmats[32 * kv + i, kv, 64 + i] = 1.0
            kmats[64 + 32 * kv + i, kv, 32 + i] = 1.0
            kmats[64 + 32 * kv + i, kv, 96 + i] = 1.0

    tri_d = nc.inline_tensor(tri.astype(ml_dtypes.bfloat16), "trimask")
    id_d = nc.inline_tensor(ident, "ident")
    on_d = nc.inline_tensor(ones1, "ones1")
    pm_d = nc.inline_tensor(
        pmats.reshape(PT, 4 * PT).astype(ml_dtypes.bfloat16), "pmats")
    km_d = nc.inline_tensor(
        kmats.reshape(PT, KVPC * PT).astype(ml_dtypes.bfloat16), "kmats")

    Exp = mybir.ActivationFunctionType.Exp
    groups = [[0, 1, 2, 3], [4, 5, 6, 7]]

    with tile.TileContext(nc) as tc, ExitStack() as ctx:
        keep = ctx.enter_context(tc.tile_pool(name="keep", bufs=1))
        # packed K cache: krp[kv] rows = [kv(a32 b32); kv(a32 b32)]
        krp0 = keep.tile([PT, S], bf16)
        krp1 = keep.tile([PT, S], bf16)
        krp = [krp0, krp1]
        v_sb = keep.tile([PT, KVPC, NT, HD + 1], bf16)   # col 64 = ones
        cos_sb = keep.tile([PT, S], bf16)
        sin_sb = keep.tile([PT, S], bf16)
        tri_sb = keep.tile([PT, SQC], bf16)
        id_sb = keep.tile([PT, PT], bf16)
        pm_sb = keep.tile([PT, 4, PT], bf16)
        km_sb = keep.tile([PT, KVPC, PT], bf16)
        wq_sb = keep.tile([PT, KT, HPC * HD], bf16)
        wk_sb = keep.tile([PT, KT, KVPC * HD], bf16)
        wv_sb = keep.tile([PT, KT, KVPC * HD], bf16)
        wo_sb = keep.tile([PT, TPC, D], bf16)

        xcache = {}
        qcps = {}
        outcs = {}

        xpool = ctx.enter_context(tc.tile_pool(name="xp", bufs=3))
        qpool = ctx.enter_context(tc.tile_pool(name="qp", bufs=2))
        qppool = ctx.enter_context(tc.tile_pool(name="qpp", bufs=2))
        kpool = ctx.enter_context(tc.tile_pool(name="kp", bufs=2))
        vtp = ctx.enter_context(tc.tile_pool(name="vtp", bufs=2))
        otp = ctx.enter_context(tc.tile_pool(name="otp", bufs=2))
        rtmp = ctx.enter_context(tc.tile_pool(name="rtmp", bufs=1))
        probs = ctx.enter_context(tc.tile_pool(name="probs", bufs=6))
        mpp = ctx.enter_context(tc.tile_pool(name="mpp", bufs=4))
        bcp = ctx.enter_context(tc.tile_pool(name="bcp", bufs=2))
        rcp = ctx.enter_context(tc.tile_pool(name="rcp", bufs=2))
        osg = ctx.enter_context(tc.tile_pool(name="osg", bufs=2))
        ysb = ctx.enter_context(tc.tile_pool(name="ysb", bufs=3))
        mw = ctx.enter_context(tc.tile_pool(name="mw", bufs=2, space="PSUM"))
        sps = ctx.enter_context(tc.tile_pool(name="sps", bufs=4, space="PSUM"))
        aps = ctx.enter_context(tc.tile_pool(name="aps", bufs=2, space="PSUM"))

        def load_x(c):
            if c >= NCHUNK or c in xcache:
                return
            xt = xpool.tile([PT, KT, SQC], bf16, tag="xt", name=f"xt{c}")
            nc.sync.dma_start(
                out=xt[:, 0:KT // 2, :],
                in_=xT_d[c].rearrange("p (k n) -> p k n", k=KT)
                [:, 0:KT // 2, :])
            nc.sync.dma_start(
                out=xt[:, KT // 2:, :],
                in_=xT_d[c].rearrange("p (k n) -> p k n", k=KT)
                [:, KT // 2:, :])
            xcache[c] = xt

        # startup loads: wq (halves) and x0 first so q-proj starts early
        wq_v = wq_d.ap().rearrange("p (k n) -> p k n", k=KT)
        nc.sync.dma_start(out=wq_sb[:, 0:KT // 2, :], in_=wq_v[:, 0:KT // 2, :])
        load_x(0)
        nc.sync.dma_start(out=wq_sb[:, KT // 2:, :], in_=wq_v[:, KT // 2:, :])
        nc.sync.dma_start(out=wk_sb[:],
                          in_=wk_d.ap().rearrange("p (k n) -> p k n", k=KT))
        nc.sync.dma_start(out=wv_sb[:],
                          in_=wv_d.ap().rearrange("p (k n) -> p k n", k=KT))
        nc.sync.dma_start(out=cos_sb[:], in_=cos_d[:])
        nc.sync.dma_start(out=sin_sb[:], in_=sin_d[:])
        nc.sync.dma_start(out=tri_sb[:], in_=tri_d[:])
        nc.sync.dma_start(out=id_sb[:], in_=id_d[:])
        nc.sync.dma_start(out=pm_sb[:],
                          in_=pm_d.ap().rearrange("p (j n) -> p j n", j=4))
        nc.sync.dma_start(out=km_sb[:],
                          in_=km_d.ap().rearrange("p (j n) -> p j n", j=KVPC))
        # ones column of v (every (kv, t) slot)
        ones_src = bass.AP(tensor=on_d.ap().tensor, offset=0,
                           ap=[[1, PT], [0, KVPC * NT], [1, 1]])
        vcol = v_sb[:, :, :, HD:HD + 1]
        ones_dst = bass.AP(tensor=vcol.tensor, offset=vcol.offset,
                           ap=[list(vcol.ap[0]), [HD + 1, KVPC * NT], [1, 1]])
        nc.sync.dma_start(out=ones_dst, in_=ones_src)
        load_x(1)
        nc.sync.dma_start(out=wo_sb[:],
                          in_=wo_d.ap().rearrange("p (k n) -> p k n", k=TPC))

        def rope_pair(a, b, cs, sn, nm):
            """a' = a*cos - b*sin ; b' = a*sin + b*cos (bf16, in place)."""
            t1 = rtmp.tile(a.shape, bf16, tag="t1", name=f"t1{nm}")
            t2 = rtmp.tile(a.shape, bf16, tag="t2", name=f"t2{nm}")
            t3 = rtmp.tile(a.shape, bf16, tag="t3", name=f"t3{nm}")
            nc.vector.tensor_mul(t1[:], a, cs)
            nc.vector.tensor_mul(t2[:], a, sn)
            nc.vector.tensor_mul(t3[:], b, sn)
            nc.vector.tensor_sub(a, t1[:], t3[:])
            t4 = rtmp.tile(a.shape, bf16, tag="t3", name=f"t4{nm}")
            nc.vector.tensor_mul(t4[:], b, cs)
            nc.vector.tensor_add(b, t2[:], t4[:])

        def gen_prep(c):
            """Generator: yields once per PE matmul so prep can be pumped
            as filler inside the previous chunk's attention."""
            csl = slice(c * SQC, (c + 1) * SQC)
            load_x(c)
            load_x(c + 1)          # prefetch next chunk behind this one
            xt = xcache.pop(c)

            qc = qpool.tile([PT, 4, SQC], bf16, tag="qc", name=f"qc{c}")
            kc = kpool.tile([PT, SQC], bf16, tag="kc", name=f"kc{c}")
            vtc = vtp.tile([PT, SQC], bf16, tag="vtc", name=f"vtc{c}")
            for mt in range(4):
                ps = mw.tile([PT, SQC], f32, tag="ps", name=f"qps{c}_{mt}")
                for k in range(KT):
                    nc.tensor.matmul(
                        ps[:], wq_sb[:, k, mt * PT:(mt + 1) * PT],
                        xt[:, k, :],
                        start=(k == 0), stop=(k == KT - 1))
                    yield
                nc.vector.tensor_copy(qc[:, mt, :], ps[:])
            for dst, wsb, nm in ((kc, wk_sb, "k"), (vtc, wv_sb, "v")):
                ps = mw.tile([PT, SQC], f32, tag="ps", name=f"ps{nm}{c}")
                for k in range(KT):
                    nc.tensor.matmul(
                        ps[:], wsb[:, k, :],
                        xt[:, k, :],
                        start=(k == 0), stop=(k == KT - 1))
                    yield
                nc.vector.tensor_copy(dst[:], ps[:])

            # ---- rope(c) (DVE) ------------------------------------------
            for j in range(2):
                rope_pair(qc[:, j, :], qc[:, 2 + j, :],
                          cos_sb[:, csl], sin_sb[:, csl], f"q{c}_{j}")
            # k pair: rows 0:64 / 64:128 — stage B rows to base 0 via DMA
            # (triggered from the vector queue so it never blocks others)
            bst = rtmp.tile([64, SQC], bf16, tag="t1", name=f"bst{c}")
            nc.gpsimd.dma_start(out=bst[:], in_=kc[64:128, :])
            kt1 = rtmp.tile([64, SQC], bf16, tag="t2", name=f"kt1{c}")
            kt2 = rtmp.tile([64, SQC], bf16, tag="t3", name=f"kt2{c}")
            kt3 = rtmp.tile([64, SQC], bf16, tag="t1b", name=f"kt3{c}")
            kt4 = rtmp.tile([64, SQC], bf16, tag="t2b", name=f"kt4{c}")
            nc.vector.tensor_mul(kt1[:], kc[0:64, :], cos_sb[0:64, csl])
            nc.vector.tensor_mul(kt2[:], kc[0:64, :], sin_sb[0:64, csl])
            nc.vector.tensor_mul(kt3[:], bst[:], sin_sb[0:64, csl])
            nc.vector.tensor_mul(kt4[:], bst[:], cos_sb[0:64, csl])
            nc.vector.tensor_sub(kc[0:64, :], kt1[:], kt3[:])
            kbr = rtmp.tile([64, SQC], bf16, tag="t3b", name=f"kbr{c}")
            nc.vector.tensor_add(kbr[:], kt2[:], kt4[:])
            nc.gpsimd.dma_start(out=kc[64:128, :], in_=kbr[:])

            # ---- pack(c) on the PE: qcp cols + krp via perm matmuls -----
            qcp = qppool.tile([PT, 4, SQC], bf16, tag="qcp", name=f"qcp{c}")
            qcps[c] = qcp
            for j in range(4):
                ps = mw.tile([PT, SQC], f32, tag="ps", name=f"qpp{c}_{j}")
                nc.tensor.matmul(ps[:], pm_sb[:, j % 2, :],
                                 qc[:, j // 2, :], start=True, stop=False)
                yield
                nc.tensor.matmul(ps[:], pm_sb[:, 2 + (j % 2), :],
                                 qc[:, 2 + j // 2, :], start=False, stop=True)
                yield
                nc.vector.tensor_copy(qcp[:, j, :], ps[:])
            for kv in range(KVPC):
                ps = mw.tile([PT, SQC], f32, tag="ps", name=f"kpp{c}_{kv}")
                nc.tensor.matmul(ps[:], km_sb[:, kv, :], kc[:],
                                 start=True, stop=True)
                yield
                nc.vector.tensor_copy(krp[kv][:, csl], ps[:])

            # ---- v(c): transpose vT chunk into v_sb ---------------------
            for tl in range(TPC):
                t = c * TPC + tl
                tp = mw.tile([PT, SQC], f32, tag="ps", name=f"tp{c}_{tl}")
                tpb = tp[:, 0:PT].bitcast(bf16)[:, 0:PT]
                nc.tensor.transpose(tpb,
                                    vtc[:, tl * PT:(tl + 1) * PT],
                                    id_sb[:])
                yield
                nc.vector.tensor_copy(v_sb[:, 0, t, 0:HD], tpb[:, 0:HD])
                nc.vector.tensor_copy(v_sb[:, 1, t, 0:HD], tpb[:, HD:2 * HD])

        def gen_wo(c):
            """Generator: yields once per PE matmul; wo(c) runs as filler
            inside attention(c+1)."""
            outc = outcs.pop(c)
            for tl in range(TPC):
                tt = c * TPC + tl
                yt = ysb.tile([PT, D], bf16, tag="yt", name=f"yt{c}_{tl}")
                for nk in range(4):
                    yp = mw.tile([PT, SQC], f32, tag="ps",
                                 name=f"yp{c}_{tl}_{nk}")
                    for k4 in range(4):
                        nc.tensor.matmul(
                            yp[:], outc[:, k4, tl * PT:(tl + 1) * PT],
                            wo_sb[:, k4, nk * SQC:(nk + 1) * SQC],
                            start=(k4 == 0), stop=(k4 == 3))
                        yield
                    nc.vector.tensor_copy(yt[:, nk * SQC:(nk + 1) * SQC],
                                          yp[:])
                nc.sync.dma_start(out=y_part[tt * PT:(tt + 1) * PT, :],
                                  in_=yt[:])

        def issue_rs(c):
            """RS for chunk c; the y_rs->y_out copy for c-1 rides right
            behind the trigger (RS(c-1) is complete by then, so the copy's
            wait never blocks the gpsimd queue)."""
            nc.gpsimd.collective_compute(
                "ReduceScatter", mybir.AluOpType.add,
                replica_groups=groups,
                ins=[y_part.ap()[c * SQC:(c + 1) * SQC, :]],
                outs=[y_rs.ap()[c * PT:(c + 1) * PT, :]])
            if c > 0:
                nc.gpsimd.dma_start(
                    out=y_out.ap()[(c - 1) * PT:c * PT, :],
                    in_=y_rs.ap()[(c - 1) * PT:c * PT, :])

        # ---- filler pump ------------------------------------------------
        pending = []       # [gen, on_done]

        def pump(n):
            done = 0
            while done < n and pending:
                item = pending[0]
                try:
                    next(item[0])
                    done += 1
                except StopIteration:
                    if item[1] is not None:
                        item[1]()
                    pending.pop(0)

        def drain_all():
            while pending:
                pump(1 << 20)

        def attention(c):
            qcp = qcps.pop(c)
            outc = otp.tile([PT, 4, SQC], bf16, tag="outc", name=f"outc{c}")
            outcs[c] = outc
            ntk = 4 * c + 4
            for pj in range(4):
                g = pj // 2
                avA = aps.tile([PT, SQC], f32, tag="av", name=f"avA{c}_{pj}")
                avB = aps.tile([PT, SQC], f32, tag="av", name=f"avB{c}_{pj}")
                pbq = []
                for step in range(ntk + LAG):
                    if step < ntk:
                        t = step
                        ksl = slice(t * PT, (t + 1) * PT)
                        diag = t >= 4 * c
                        off = (t - 4 * c) * PT if diag else 0
                        scA = sps.tile([PT, SQC], f32, tag="sc",
                                       name=f"scA{c}_{pj}_{t}")
                        scB = sps.tile([PT, SQC], f32, tag="sc",
                                       name=f"scB{c}_{pj}_{t}")
                        # back-to-back into disjoint row groups -> concurrent
                        nc.tensor.matmul(
                            scA[:, off:], krp[g][0:64, ksl],
                            qcp[0:64, pj, off:],
                            start=True, stop=True, tile_position=(0, 0))
                        nc.tensor.matmul(
                            scB[:, off:], krp[g][64:128, ksl],
                            qcp[64:128, pj, off:],
                            start=True, stop=True, tile_position=(64, 0))
                        pbA = probs.tile([PT, SQC], bf16, tag="pb",
                                         name=f"pbA{c}_{pj}_{t}")
                        pbB = probs.tile([PT, SQC], bf16, tag="pb",
                                         name=f"pbB{c}_{pj}_{t}")
                        nc.scalar.activation(pbA[:, off:], scA[:, off:], Exp)
                        nc.scalar.activation(pbB[:, off:], scB[:, off:], Exp)
                        if diag:
                            mpA = mpp.tile([PT, SQC], bf16, tag="mp",
                                           name=f"mpA{c}_{pj}_{t}")
                            mpB = mpp.tile([PT, SQC], bf16, tag="mp",
                                           name=f"mpB{c}_{pj}_{t}")
                            nc.vector.tensor_mul(mpA[:, off:], pbA[:, off:],
                                                 tri_sb[:, 0:SQC - off])
                            nc.vector.tensor_mul(mpB[:, off:], pbB[:, off:],
                                                 tri_sb[:, 0:SQC - off])
                            pbq.append((mpA, mpB, off))
                        else:
                            pbq.append((pbA, pbB, 0))
                    if step >= LAG:
                        t = step - LAG
                        eA, eB, off = pbq[t]
                        nc.tensor.matmul(
                            avA[0:HD + 1, off:], v_sb[:, g, t, :],
                            eA[:, off:],
                            start=(t == 0), stop=(t == ntk - 1))
                        nc.tensor.matmul(
                            avB[0:HD + 1, off:], v_sb[:, g, t, :],
                            eB[:, off:],
                            start=(t == 0), stop=(t == ntk - 1))
                    pump(PUMP)
                for qh, av in ((2 * pj, avA), (2 * pj + 1, avB)):
                    rc = rcp.tile([1, SQC], f32, tag="rc",
                                  name=f"rc{c}_{qh}")
                    nc.vector.reciprocal(rc[:], av[HD:HD + 1, :])
                    bc = bcp.tile([64, SQC], f32, tag="bc",
                                  name=f"bc{c}_{qh}")
                    nc.gpsimd.partition_broadcast(bc[:], rc[:])
                    dst = outc[(qh % 2) * HD:(qh % 2 + 1) * HD, qh // 2, :]
                    if qh % 2 == 0:
                        nc.vector.tensor_mul(dst, av[0:HD, :], bc[:])
                    else:
                        st = osg.tile([64, SQC], bf16, tag="st",
                                      name=f"st{c}_{qh}")
                        nc.vector.tensor_mul(st[:], av[0:HD, :], bc[:])
                        nc.sync.dma_start(out=dst, in_=st[:])
                pump(PUMP)

        # ---- main pipeline ----------------------------------------------
        for _ in gen_prep(0):
            pass
        for c in range(NCHUNK):
            # wo(c-1) first (its RS gates the collective timeline), then the
            # next chunks' preps
            if c > 0:
                pending.append([gen_wo(c - 1),
                                (lambda cc: lambda: issue_rs(cc))(c - 1)])
            if c + 1 < NCHUNK and (c + 1) not in qcps:
                pending.append([gen_prep(c + 1), None])
            attention(c)
            # prep(c+1) must be complete before attention(c+1) starts
            drain_all()
        for _ in gen_wo(NCHUNK - 1):
            pass
        issue_rs(NCHUNK - 1)
        nc.gpsimd.dma_start(
            out=y_out.ap()[(NCHUNK - 1) * PT:NCHUNK * PT, :],
            in_=y_rs.ap()[(NCHUNK - 1) * PT:NCHUNK * PT, :])

    nc.compile()
    return nc


@functools.lru_cache(maxsize=2)
def _get_program():
    return _build_program()


def _host_inputs(x, wq, wk, wv, wo, cos, sin):
    """Build the 8 per-core input maps (all partition-major contiguous)."""
    import ml_dtypes

    perm_q = np.empty(HPC * HD, np.int64)
    for rho in range(HPC * HD):
        blk, rem = divmod(rho, HPC * HD // 2)
        h, i = divmod(rem, 32)
        perm_q[rho] = h * HD + 2 * i + blk
    perm_k = np.empty(KVPC * HD, np.int64)
    for rho in range(KVPC * HD):
        blk, rem = divmod(rho, KVPC * HD // 2)
        kv, i = divmod(rem, 32)
        perm_k[rho] = kv * HD + 2 * i + blk

    reps = np.tile(np.arange(32), 4)
    cosr = np.ascontiguousarray(cos.T[reps]).astype(ml_dtypes.bfloat16)
    sinr = np.ascontiguousarray(sin.T[reps]).astype(ml_dtypes.bfloat16)

    def pmajor(w):
        """[D_in, M] -> [128, KT_w * M] with [p, k*M+m] = w[k*128+p, m]."""
        kt = w.shape[0] // PT
        return np.ascontiguousarray(
            w.reshape(kt, PT, w.shape[1]).transpose(1, 0, 2)
            .reshape(PT, kt * w.shape[1])).astype(ml_dtypes.bfloat16)

    xts = []
    for b in range(B):
        # [c, p, k*512+n] = x[b, c*512+n, k*128+p]
        xb = x[b].reshape(NCHUNK, SQC, KT, PT).transpose(0, 3, 2, 1)
        xts.append(np.ascontiguousarray(
            xb.reshape(NCHUNK, PT, KT * SQC)).astype(ml_dtypes.bfloat16))

    scale = np.float32(1.0 / np.sqrt(HD))
    in_maps = []
    for core in range(N_CORES):
        b, hg = divmod(core, GROUP)
        qcols = slice(hg * HPC * HD, (hg + 1) * HPC * HD)
        kcols = slice(hg * KVPC * HD, (hg + 1) * KVPC * HD)
        wq_c = (wq[:, qcols] * scale)[:, perm_q]
        wk_c = wk[:, kcols][:, perm_k]
        wv_c = wv[:, kcols]
        wo_c = wo[qcols, :]
        in_maps.append({
            "xt": xts[b],
            "wq": pmajor(wq_c),
            "wk": pmajor(wk_c),
            "wv": pmajor(wv_c),
            "wo": pmajor(wo_c),
            "cosr": cosr,
            "sinr": sinr,
        })
    return in_maps


def _assemble(results):
    """results[core]["y"]: [S/GROUP, D] bf16; chunk c rows [c*128:(c+1)*128]
    hold tokens c*512 + r*128 .. +128 for group rank r."""
    out = np.empty((B, S, D), np.float32)
    for b in range(B):
        for r in range(GROUP):
            y = np.asarray(results[b * GROUP + r]["y"], np.float32)
            for c in range(NCHUNK):
                rows = slice(c * SQC + r * PT, c * SQC + (r + 1) * PT)
                out[b, rows, :] = y[c * PT:(c + 1) * PT, :]
    return out


def _is_causal(mask):
    if mask.shape != (S, S):
        return False
    expect = np.where(np.tril(np.ones((S, S), bool)), np.float32(0.0),
                      np.float32(NEG))
    return np.array_equal(mask, expect)


def _numpy_fallback(x, wq, wk, wv, wo, cos, sin, mask):
    """Exact reference math on host (only used if mask isn't causal)."""
    xq = (x @ wq).reshape(B, S, H, HD)
    xk = (x @ wk).reshape(B, S, KVH, HD)
    xv = (x @ wv).reshape(B, S, KVH, HD)

    def rope(t):
        tr = t.reshape(*t.shape[:-1], HD // 2, 2)
        a, b = tr[..., 0], tr[..., 1]
        c = cos[None, :, None, :]
        s_ = sin[None, :, None, :]
        out = np.stack([a * c - b * s_, a * s_ + b * c], axis=-1)
        return out.reshape(t.shape)

    xq, xk = rope(xq), rope(xk)
    xk = np.repeat(xk, H // KVH, axis=2)
    xv = np.repeat(xv, H // KVH, axis=2)
    q = xq.transpose(0, 2, 1, 3)
    k = xk.transpose(0, 2, 1, 3)
    v = xv.transpose(0, 2, 1, 3)
    sc = np.einsum("bhqd,bhkd->bhqk", q, k) / np.sqrt(np.float32(HD))
    sc = sc + mask[None, None]
    sc = sc - sc.max(-1, keepdims=True)
    p = np.exp(sc)
    p /= p.sum(-1, keepdims=True)
    out = np.einsum("bhqk,bhkd->bhqd", p, v)
    out = out.transpose(0, 2, 1, 3).reshape(B, S, H * HD)
    return (out @ wo).astype(np.float32)


def _ensure_ntff_hook():
    """Provide antenv.axon_hooks (missing on this image) so trace=True works."""
    try:
        from antenv.axon_hooks import get_axon_ntff_profile_hook  # noqa: F401
        return True
    except ImportError:
        pass
    try:
        import types
        import antenv
        from trn_agent_boot.trn_boot import _ntff_profile_via_ctypes

        mod = types.ModuleType("antenv.axon_hooks")
        _state = {"hook": None}
        mod.set_axon_ntff_profile_hook = \
            lambda h: _state.__setitem__("hook", h)
        mod.get_axon_ntff_profile_hook = lambda: _state["hook"]
        sys.modules["antenv.axon_hooks"] = mod
        antenv.axon_hooks = mod
        mod.set_axon_ntff_profile_hook(
            _ntff_profile_via_ctypes("/opt/axon/libaxon_pjrt.so"))
        return mod.get_axon_ntff_profile_hook() is not None
    except Exception:
        return False


def kernel(x, wq, wk, wv, wo, cos, sin, mask):
    x = np.asarray(x, np.float32)
    wq = np.asarray(wq, np.float32)
    wk = np.asarray(wk, np.float32)
    wv = np.asarray(wv, np.float32)
    wo = np.asarray(wo, np.float32)
    cos = np.asarray(cos, np.float32)
    sin = np.asarray(sin, np.float32)
    mask = np.asarray(mask, np.float32)

    if not _is_causal(mask):
        return _numpy_fallback(x, wq, wk, wv, wo, cos, sin, mask)

    from concourse.bass_utils import run_bass_kernel_spmd

    nc = _get_program()
    in_maps = _host_inputs(x, wq, wk, wv, wo, cos, sin)
    trace = bool(int(os.environ.get("ATTN_TRACE", "0")))
    if trace and not _ensure_ntff_hook():
        trace = False
    res = run_bass_kernel_spmd(nc, in_maps, core_ids=list(range(N_CORES)),
                               trace=trace)
    if trace:
        kernel.last_exec_time_ns = res.exec_time_ns
        kernel.last_results = res
    return _assemble(res.results)
